# revision 27
# baseline (speedup 1.0000x reference)
"""DiffAttn kernel for 8 Trainium2 NeuronCores.

Problem: out = softmax(Q1 K1^T / sqrt(d)) V - lam * softmax(Q2 K2^T / sqrt(d)) V
with Q = X W_q, K = X W_k, V = X W_v;  X [2, 4096, 1024], W [1024, 128], d = 64.

Sharding: 8 cores = (batch b, query-chunk qc) with b = core // 4, qc = core % 4.
Each core receives its batch's X TRANSPOSED ([din, seq]) with the seq columns
rolled so its 1024 query columns come first (attention is permutation-invariant
over keys).  Feeding X^T from the host removes every on-device transpose of X:
projections contract din directly.  Each core computes K^T/V^T for the full
(rolled) sequence and Q^T for its 1024 queries, then two-branch flash attention
without max-subtraction (scores ~N(0,1); exp is safe), normalizing at the end.

Engine split: PE does projections + scores + P@V only.  ACT does exp
(f32 PSUM -> bf16 SBUF).  DVE accumulates the exp tiles (bf16 partials, fp32
fold) for the softmax row sums; Pool drains PSUM->SBUF and performs the final
128-partition reduction (partition_all_reduce).  Because one exp instruction
(>=1042 ns) is slower than the four PE matmuls per key tile (852 ns), pure
attention stretches would stall the PE on the 2-deep score-PSUM rotation; the
schedule therefore (a) interleaves projection-chain matmuls between attention
units inside the seq-tile loop, and (b) defers the second query-tile's P@V
matmuls to the tail, feeding them from SBUF-buffered bf16 exp tiles so the
tail is pure PE work.  Output is written transposed [2d, 1024] and fixed up on
the host.
"""

import sys

if '/opt/trn_rl_repo' not in sys.path:
    sys.path.insert(0, '/opt/trn_rl_repo')

import numpy as np

B, S, DIN, D = 2, 4096, 1024, 64
TD = 2 * D            # 128: both branches' head dims, packed on partitions
NQ = S // 4           # 1024 query rows per core
ST = 512              # seq tile (projection granularity)
NST = S // ST         # 8
QT = 512              # query tile in attention
NQT = NQ // QT        # 2
KT = 128              # key tile in attention
NKT = S // KT         # 32
NDC = DIN // 128      # 8 contraction chunks
LOOPK = 28            # key tiles scored inside the seq-tile loop (lag 1)


def build_nc():
    import concourse.bacc as bacc
    import concourse.mybir as mybir
    import concourse.bass_isa as bass_isa
    from concourse.tile import TileContext
    from concourse.masks import make_identity

    F32 = mybir.dt.float32
    F32R = mybir.dt.float32r
    BF16 = mybir.dt.bfloat16
    AF = mybir.ActivationFunctionType

    nc = bacc.Bacc("TRN2", target_bir_lowering=False)
    XT_t = nc.dram_tensor("XT", [DIN, S], F32, kind="ExternalInput")
    Wq_t = nc.dram_tensor("Wq", [DIN, TD], F32, kind="ExternalInput")
    Wk_t = nc.dram_tensor("Wk", [DIN, TD], F32, kind="ExternalInput")
    Wv_t = nc.dram_tensor("Wv", [DIN, TD], F32, kind="ExternalInput")
    lam_t = nc.dram_tensor("lam", [1, 1], F32, kind="ExternalInput")
    out_t = nc.dram_tensor("out", [TD, NQ], F32, kind="ExternalOutput")

    with TileContext(nc) as tc:
        with tc.tile_pool(name="consts", bufs=1) as consts, \
             tc.tile_pool(name="kv", bufs=1) as kv:
            w_sb = {}
            w_dram = {"wk": Wk_t, "wv": Wv_t, "wq": Wq_t}
            for name in ("wk", "wv", "wq"):
                w_sb[name] = consts.tile([128, NDC, TD], F32R, tag=name,
                                         name=name)

            def load_w(name, q, n=1):
                qc = NDC // 4
                nc.sync.dma_start(
                    out=w_sb[name][:, q * qc:(q + n) * qc, :],
                    in_=w_dram[name].ap()[q * 256:(q + n) * 256, :]
                    .rearrange("(c p) n -> p c n", p=128).bitcast(F32R))
            load_w("wk", 0)
            ident = consts.tile([128, 128], F32, tag="ident")
            make_identity(nc, ident)
            identr = consts.tile([128, 128], F32R, tag="identr")
            nc.scalar.copy(out=identr, in_=ident)
            lam_sb = consts.tile([128, 1], F32, tag="lam")

            # resident projections + the qt1 exp-tile store
            kT = kv.tile([128, S], F32R, tag="kT")         # K^T [2d, S]
            qT = kv.tile([128, NQ], F32R, tag="qT")        # Q^T [2d, NQ]
            v_sb = kv.tile([128, NKT, TD], BF16, tag="v")  # V natural
            e1_sb = kv.tile([128, NKT, 2, QT], BF16, tag="e1")  # qt1 exps

            with tc.tile_pool(name="xt", bufs=2) as xt_pool, \
                 tc.tile_pool(name="vts", bufs=2) as vt_pool, \
                 tc.tile_pool(name="sps", bufs=2, space="PSUM") as sps, \
                 tc.tile_pool(name="ops0", bufs=1, space="PSUM") as ops0, \
                 tc.tile_pool(name="e0", bufs=8) as e0_pool, \
                 tc.tile_pool(name="racc", bufs=1) as racc_pool, \
                 tc.tile_pool(name="epi", bufs=1) as epi_pool, \
                 tc.tile_pool(name="osb", bufs=1) as osb:

                racc = [[racc_pool.tile([128, 2, QT], BF16, tag=f"a{q}{j}",
                                        name=f"a{q}{j}")
                         for j in range(2)] for q in range(NQT)]
                rpart = [racc_pool.tile([128, 2, QT], F32, tag=f"rp{q}",
                                        name=f"rp{q}") for q in range(NQT)]
                o_ps = [None, None]
                o_ps[0] = ops0.tile([128, 2, QT], F32, tag="o0", name="o0")
                e0_live = {}
                vT = [None] * NST

                def chain_pieces(wname, dst, xt, st, pps, tag):
                    """Projection chain for one seq tile as 4 interleavable
                    2-matmul pieces; the final piece issues the Pool copy."""
                    ps = pps.tile([128, ST], F32, tag=tag, name="ps")

                    def piece(i):
                        for dc in (2 * i, 2 * i + 1):
                            nc.tensor.matmul(
                                ps, w_sb[wname][:, dc, :], xt[:, dc, :],
                                start=(dc == 0), stop=(dc == NDC - 1))
                        if i == 3:
                            nc.vector.tensor_copy(
                                dst[:, st * ST:(st + 1) * ST]
                                if dst is not None else vT[st], ps)
                    return [lambda i=i: piece(i) for i in range(4)]

                def vtrans(st, pps):
                    # V natural blocks for seq tile st
                    def piece():
                        vp4 = pps.tile([128, 4, TD], F32R, tag="pb",
                                       name="vp4")
                        for j in range(4):
                            nc.tensor.transpose(
                                vp4[:, j, :], vT[st][:, j * 128:(j + 1) * 128],
                                identr)
                        nc.vector.tensor_copy(v_sb[:, st * 4:st * 4 + 4, :],
                                               vp4)
                    return [piece]

                def sc_exp(qt, i, kt):
                    """Scores + exp (+ row-sum add) for one (qt, key tile)."""
                    q0 = qt * QT
                    k0 = kt * KT
                    s12 = sps.tile([128, 2, QT], F32, tag="s", name="s12")
                    nc.tensor.matmul(
                        s12[:, 0, :], kT[0:64, k0:k0 + KT],
                        qT[0:64, q0:q0 + QT], start=True, stop=True)
                    nc.tensor.matmul(
                        s12[:, 1, :], kT[64:128, k0:k0 + KT],
                        qT[64:128, q0:q0 + QT], start=True, stop=True)
                    if qt == 0:
                        e12 = e0_pool.tile([128, 2, QT], BF16, tag="e")
                    else:
                        e12 = e1_sb[:, kt, :, :]
                    nc.scalar.activation(
                        out=e12, in_=s12, func=AF.Exp, scale=0.125)
                    # block-grouped row-sum accumulation: 4 groups of 8
                    # key tiles in 2 alternating bf16 tiles, folded into the
                    # fp32 partial as each group completes, so only one fold
                    # remains after the last exp
                    a = racc[qt][(i // 8) % 2]
                    if i % 8 == 0:
                        nc.vector.tensor_copy(a, e12)
                    else:
                        nc.vector.tensor_add(a, a, e12)
                    if i == 15:
                        nc.vector.tensor_add(rpart[qt], racc[qt][0],
                                             racc[qt][1])
                    elif i == 23:
                        nc.vector.tensor_add(rpart[qt], rpart[qt],
                                             racc[qt][0])
                    return e12

                def out_mm(qt, i, kt, e12):
                    first, last = (i == 0), (i == NKT - 1)
                    o12 = o_ps[qt]
                    nc.tensor.matmul(o12[:, 0, :], v_sb[:, kt, :],
                                     e12[:, 0, :], start=first, stop=last)
                    nc.tensor.matmul(o12[:, 1, :], v_sb[:, kt, :],
                                     e12[:, 1, :], start=first, stop=last)

                def epilogue_r(qt):
                    # row-sum reduction; only needs racc, not o
                    rinvs = []
                    for h in range(2):
                        HQ = QT // 2
                        hs = slice(h * HQ, (h + 1) * HQ)
                        rtot = epi_pool.tile([128, 2, HQ], F32,
                                             tag=f"rt{h}", name="rt")
                        nc.vector.tensor_add(rtot, rpart[qt][:, :, hs],
                                             racc[qt][1][:, :, hs])
                        rall = epi_pool.tile([128, 2, HQ], F32,
                                             tag=f"ra{h}", name="ra")
                        nc.gpsimd.partition_all_reduce(
                            rall, rtot, channels=128,
                            reduce_op=bass_isa.ReduceOp.add)
                        rinv = epi_pool.tile([128, 2, HQ], F32,
                                             tag=f"ri{h}", name="ri")
                        nc.vector.reciprocal(rinv, rall)
                        rinvs.append(rinv)
                    return rinvs

                def epilogue_o(qt, rinvs):
                    # all on DVE: GPSIMD cannot touch PSUM on TRN2, and
                    # qt0's chain hides under the long qt1 P@V stretch anyway.
                    # The very last chunk (qt1, second half) runs in quarters
                    # so its first DMA overlaps the remaining DVE compute.
                    eng = nc.vector
                    o12 = o_ps[qt]
                    nch = 2
                    for h in range(nch):
                        HQ = QT // nch
                        hs = slice(h * HQ, (h + 1) * HQ)
                        rinv = rinvs[h // (nch // 2)]
                        HR = QT // 2
                        rs = slice((h * HQ) % HR, (h * HQ) % HR + HQ)
                        t1 = osb.tile([128, HQ], F32, tag=f"t1{qt}{h}",
                                      name="t1")
                        t2 = osb.tile([128, HQ], F32, tag=f"t2{qt}{h}",
                                      name="t2")
                        eng.tensor_mul(t1, o12[:, 0, hs], rinv[:, 0, rs])
                        eng.scalar_tensor_tensor(
                            out=t2, in0=o12[:, 1, hs], scalar=lam_sb,
                            in1=rinv[:, 1, rs], op0=mybir.AluOpType.mult,
                            op1=mybir.AluOpType.mult)
                        ob = osb.tile([128, HQ], F32, tag=f"ob{qt}{h}",
                                      name="ob")
                        eng.tensor_sub(ob, t1, t2)
                        q0 = qt * QT + h * HQ
                        nc.sync.dma_start(
                            out=out_t.ap()[:, q0:q0 + HQ], in_=ob)

                # ---- seq-tile loop: projections + attention, interleaved ----
                with tc.tile_pool(name="pps", bufs=1, space="PSUM") as pps:
                    for st in range(NST):
                        xt = xt_pool.tile([128, NDC, ST], F32R, tag="xt")
                        s0 = st * ST
                        nh = 8 if st == 0 else 2
                        for h in range(nh):
                            w = DIN // nh
                            nc.sync.dma_start(
                                out=xt[:, h * (NDC // nh):
                                       (h + 1) * (NDC // nh), :],
                                in_=XT_t.ap()[h * w:(h + 1) * w, s0:s0 + ST]
                                .rearrange("(c p) s -> p c s", p=128)
                                .bitcast(F32R))
                            if st == 0 and h == 0:
                                load_w("wk", 1)
                            if st == 0 and h == 2:
                                load_w("wk", 2, 2)
                            if st == 0 and h == 3:
                                load_w("wv", 0, 2)
                            if st == 0 and h == 5:
                                load_w("wv", 2, 2)
                            if st == 0 and h == 7:
                                load_w("wq", 0, 4)
                                nc.gpsimd.dma_start(
                                    out=lam_sb,
                                    in_=lam_t.ap().partition_broadcast(128))
                        vT[st] = vt_pool.tile([128, ST], F32R, tag="vT",
                                              name="vTs")

                        kf = chain_pieces("wk", kT, xt, st, pps, "pa")
                        vf = chain_pieces("wv", None, xt, st, pps, "pb")
                        qf = (chain_pieces("wq", qT, xt, st, pps, "pa")
                              if st < 2 else [])
                        vt_f = vtrans(st - 1, pps) if st >= 1 else []
                        if st == 0:
                            # K, then Q (pa reuse covered by V pieces)
                            for f in (kf[0], kf[1], kf[2], kf[3], vf[0],
                                      vf[1], qf[0], qf[1], qf[2], qf[3],
                                      vf[2], vf[3]):
                                f()
                            continue
                        if st == 1:
                            # qt0 scores (kts 0-3) only need K(st0)/Q(st0);
                            # qt1 scores wait for this tile's Q chain
                            sched = [
                                None, vt_f[0], None, qf[0], None, qf[1],
                                None, qf[2], qf[3], kf[0], kf[1],
                                'q1', kf[2], 'q1', kf[3], 'q1', vf[0],
                                'q1', vf[1], vf[2], vf[3]]
                            q0s = iter(range(4))
                            q1s = iter(range(4))
                            for f in sched:
                                if f is None:
                                    j = next(q0s)
                                    e0_live[j] = sc_exp(0, j, j)
                                elif f == 'q1':
                                    j = next(q1s)
                                    sc_exp(1, j, j)
                                else:
                                    f()
                            continue
                        # st >= 2: scores at lag 1, qt0 P@V at lag 2
                        # (qt1 P@V deferred to the tail); projection pieces
                        # fill the PE between attention units
                        filler = vt_f + kf + vf
                        fi = iter(filler)
                        next(fi, lambda: None)()
                        next(fi, lambda: None)()
                        for j in range(4):
                            kt = (st - 1) * 4 + j
                            ktp = (st - 2) * 4 + j
                            e0_live[kt] = sc_exp(0, kt, kt)
                            next(fi, lambda: None)()
                            sc_exp(1, kt, kt)
                            next(fi, lambda: None)()
                            out_mm(0, ktp, ktp, e0_live.pop(ktp))
                        for f in fi:
                            f()
                    # last V transpose
                    for f in vtrans(NST - 1, pps):
                        f()

                # ---- tail: kts 28..31 scores/exp for both qts, with the
                # deferred qt1 P@V matmuls as pure-PE filler ----
                with tc.tile_pool(name="ops1", bufs=1, space="PSUM") as ops1:
                    o_ps[1] = ops1.tile([128, 2, QT], F32, tag="o1",
                                        name="o1")
                    fi = iter(range(16))  # first deferred qt1 outs
                    for kt in range(LOOPK, NKT):
                        e0_live[kt] = sc_exp(0, kt, kt)
                        for _ in range(2):
                            k2 = next(fi, None)
                            if k2 is not None:
                                out_mm(1, k2, k2, e1_sb[:, k2, :, :])
                        sc_exp(1, kt, kt)
                        for _ in range(2):
                            k2 = next(fi, None)
                            if k2 is not None:
                                out_mm(1, k2, k2, e1_sb[:, k2, :, :])
                    # row-sum reductions run on DVE/Pool while the PE
                    # finishes the P@V matmuls
                    rinvs0 = epilogue_r(0)
                    rinvs1 = epilogue_r(1)
                    for kt in range(LOOPK - 4, NKT):
                        out_mm(0, kt, kt, e0_live.pop(kt))
                    epilogue_o(0, rinvs0)
                    # a long pure-PE stretch: every epilogue dependency
                    # (reciprocals, qt0 output chain) resolves under it
                    for kt in range(16, NKT):
                        out_mm(1, kt, kt, e1_sb[:, kt, :, :])
                    epilogue_o(1, rinvs1)

    nc.compile()
    return nc


_NC_CACHE = None


def kernel(X, W_q, W_k, W_v, lam):
    global _NC_CACHE
    from concourse.bass_utils import run_bass_kernel_spmd

    X = np.asarray(X, dtype=np.float32)
    W_q = np.asarray(W_q, dtype=np.float32)
    W_k = np.asarray(W_k, dtype=np.float32)
    W_v = np.asarray(W_v, dtype=np.float32)
    lam_arr = np.asarray(lam, dtype=np.float32).reshape(1, 1)

    if _NC_CACHE is None:
        _NC_CACHE = build_nc()
    nc = _NC_CACHE

    in_maps = []
    for c in range(8):
        b, qc = divmod(c, 4)
        qs = qc * NQ
        XTb = X[b].T  # [DIN, S]
        XTc = np.ascontiguousarray(
            np.concatenate([XTb[:, qs:], XTb[:, :qs]], axis=1))
        in_maps.append({"XT": XTc, "Wq": W_q, "Wk": W_k, "Wv": W_v,
                        "lam": lam_arr})

    res = run_bass_kernel_spmd(nc, in_maps, core_ids=list(range(8)))

    out = np.empty((B, S, TD), dtype=np.float32)
    for c in range(8):
        b, qc = divmod(c, 4)
        qs = qc * NQ
        out[b, qs:qs + NQ] = res.results[c]["out"].T
    return out


# revision 28
# speedup vs baseline: 1.0037x; 1.0037x over previous
"""DiffAttn kernel for 8 Trainium2 NeuronCores.

Problem: out = softmax(Q1 K1^T / sqrt(d)) V - lam * softmax(Q2 K2^T / sqrt(d)) V
with Q = X W_q, K = X W_k, V = X W_v;  X [2, 4096, 1024], W [1024, 128], d = 64.

Sharding: 8 cores = (batch b, query-chunk qc) with b = core // 4, qc = core % 4.
Each core receives its batch's X TRANSPOSED ([din, seq]) with the seq columns
rolled so its 1024 query columns come first (attention is permutation-invariant
over keys).  Feeding X^T from the host removes every on-device transpose of X:
projections contract din directly.  Each core computes K^T/V^T for the full
(rolled) sequence and Q^T for its 1024 queries, then two-branch flash attention
without max-subtraction (scores ~N(0,1); exp is safe), normalizing at the end.

Engine split: PE does projections + scores + P@V only.  ACT does exp
(f32 PSUM -> bf16 SBUF).  DVE accumulates the exp tiles (bf16 partials, fp32
fold) for the softmax row sums; Pool drains PSUM->SBUF and performs the final
128-partition reduction (partition_all_reduce).  Because one exp instruction
(>=1042 ns) is slower than the four PE matmuls per key tile (852 ns), pure
attention stretches would stall the PE on the 2-deep score-PSUM rotation; the
schedule therefore (a) interleaves projection-chain matmuls between attention
units inside the seq-tile loop, and (b) defers the second query-tile's P@V
matmuls to the tail, feeding them from SBUF-buffered bf16 exp tiles so the
tail is pure PE work.  Output is written transposed [2d, 1024] and fixed up on
the host.
"""

import sys

if '/opt/trn_rl_repo' not in sys.path:
    sys.path.insert(0, '/opt/trn_rl_repo')

import numpy as np

B, S, DIN, D = 2, 4096, 1024, 64
TD = 2 * D            # 128: both branches' head dims, packed on partitions
NQ = S // 4           # 1024 query rows per core
ST = 512              # seq tile (projection granularity)
NST = S // ST         # 8
QT = 512              # query tile in attention
NQT = NQ // QT        # 2
KT = 128              # key tile in attention
NKT = S // KT         # 32
NDC = DIN // 128      # 8 contraction chunks
LOOPK = 28            # key tiles scored inside the seq-tile loop (lag 1)


def build_nc():
    import concourse.bacc as bacc
    import concourse.mybir as mybir
    import concourse.bass_isa as bass_isa
    from concourse.tile import TileContext
    from concourse.masks import make_identity

    F32 = mybir.dt.float32
    F32R = mybir.dt.float32r
    BF16 = mybir.dt.bfloat16
    AF = mybir.ActivationFunctionType

    nc = bacc.Bacc("TRN2", target_bir_lowering=False)
    XT_t = nc.dram_tensor("XT", [DIN, S], F32, kind="ExternalInput")
    Wq_t = nc.dram_tensor("Wq", [DIN, TD], F32, kind="ExternalInput")
    Wk_t = nc.dram_tensor("Wk", [DIN, TD], F32, kind="ExternalInput")
    Wv_t = nc.dram_tensor("Wv", [DIN, TD], F32, kind="ExternalInput")
    lam_t = nc.dram_tensor("lam", [1, 1], F32, kind="ExternalInput")
    out_t = nc.dram_tensor("out", [TD, NQ], F32, kind="ExternalOutput")

    with TileContext(nc) as tc:
        with tc.tile_pool(name="consts", bufs=1) as consts, \
             tc.tile_pool(name="kv", bufs=1) as kv:
            w_sb = {}
            w_dram = {"wk": Wk_t, "wv": Wv_t, "wq": Wq_t}
            for name in ("wk", "wv", "wq"):
                w_sb[name] = consts.tile([128, NDC, TD], F32R, tag=name,
                                         name=name)

            def load_w(name, q, n=1):
                qc = NDC // 4
                nc.sync.dma_start(
                    out=w_sb[name][:, q * qc:(q + n) * qc, :],
                    in_=w_dram[name].ap()[q * 256:(q + n) * 256, :]
                    .rearrange("(c p) n -> p c n", p=128).bitcast(F32R))
            load_w("wk", 0, 2)
            ident = consts.tile([128, 128], F32, tag="ident")
            make_identity(nc, ident)
            identr = consts.tile([128, 128], F32R, tag="identr")
            nc.scalar.copy(out=identr, in_=ident)
            lam_sb = consts.tile([128, 1], F32, tag="lam")

            # resident projections + the qt1 exp-tile store
            kT = kv.tile([128, S], F32R, tag="kT")         # K^T [2d, S]
            qT = kv.tile([128, NQ], F32R, tag="qT")        # Q^T [2d, NQ]
            v_sb = kv.tile([128, NKT, TD], BF16, tag="v")  # V natural
            e1_sb = kv.tile([128, NKT, 2, QT], BF16, tag="e1")  # qt1 exps

            with tc.tile_pool(name="xt", bufs=2) as xt_pool, \
                 tc.tile_pool(name="vts", bufs=2) as vt_pool, \
                 tc.tile_pool(name="sps", bufs=2, space="PSUM") as sps, \
                 tc.tile_pool(name="ops0", bufs=1, space="PSUM") as ops0, \
                 tc.tile_pool(name="e0", bufs=8) as e0_pool, \
                 tc.tile_pool(name="racc", bufs=1) as racc_pool, \
                 tc.tile_pool(name="epi", bufs=1) as epi_pool, \
                 tc.tile_pool(name="osb", bufs=1) as osb:

                racc = [[racc_pool.tile([128, 2, QT], BF16, tag=f"a{q}{j}",
                                        name=f"a{q}{j}")
                         for j in range(2)] for q in range(NQT)]
                rpart = [racc_pool.tile([128, 2, QT], F32, tag=f"rp{q}",
                                        name=f"rp{q}") for q in range(NQT)]
                o_ps = [None, None]
                o_ps[0] = ops0.tile([128, 2, QT], F32, tag="o0", name="o0")
                e0_live = {}
                vT = [None] * NST

                def chain_pieces(wname, dst, xt, st, pps, tag):
                    """Projection chain for one seq tile as 4 interleavable
                    2-matmul pieces; the final piece issues the Pool copy."""
                    ps = pps.tile([128, ST], F32, tag=tag, name="ps")

                    def piece(i):
                        for dc in (2 * i, 2 * i + 1):
                            nc.tensor.matmul(
                                ps, w_sb[wname][:, dc, :], xt[:, dc, :],
                                start=(dc == 0), stop=(dc == NDC - 1))
                        if i == 3:
                            nc.vector.tensor_copy(
                                dst[:, st * ST:(st + 1) * ST]
                                if dst is not None else vT[st], ps)
                    return [lambda i=i: piece(i) for i in range(4)]

                def vtrans(st, pps):
                    # V natural blocks for seq tile st
                    def piece():
                        vp4 = pps.tile([128, 4, TD], F32R, tag="pb",
                                       name="vp4")
                        for j in range(4):
                            nc.tensor.transpose(
                                vp4[:, j, :], vT[st][:, j * 128:(j + 1) * 128],
                                identr)
                        nc.vector.tensor_copy(v_sb[:, st * 4:st * 4 + 4, :],
                                               vp4)
                    return [piece]

                def sc_exp(qt, i, kt):
                    """Scores + exp (+ row-sum add) for one (qt, key tile)."""
                    q0 = qt * QT
                    k0 = kt * KT
                    s12 = sps.tile([128, 2, QT], F32, tag="s", name="s12")
                    nc.tensor.matmul(
                        s12[:, 0, :], kT[0:64, k0:k0 + KT],
                        qT[0:64, q0:q0 + QT], start=True, stop=True)
                    nc.tensor.matmul(
                        s12[:, 1, :], kT[64:128, k0:k0 + KT],
                        qT[64:128, q0:q0 + QT], start=True, stop=True)
                    if qt == 0:
                        e12 = e0_pool.tile([128, 2, QT], BF16, tag="e")
                    else:
                        e12 = e1_sb[:, kt, :, :]
                    nc.scalar.activation(
                        out=e12, in_=s12, func=AF.Exp, scale=0.125)
                    # block-grouped row-sum accumulation: 4 groups of 8
                    # key tiles in 2 alternating bf16 tiles, folded into the
                    # fp32 partial as each group completes, so only one fold
                    # remains after the last exp
                    a = racc[qt][(i // 8) % 2]
                    if i % 8 == 0:
                        nc.vector.tensor_copy(a, e12)
                    else:
                        nc.vector.tensor_add(a, a, e12)
                    if i == 15:
                        nc.vector.tensor_add(rpart[qt], racc[qt][0],
                                             racc[qt][1])
                    elif i == 23:
                        nc.vector.tensor_add(rpart[qt], rpart[qt],
                                             racc[qt][0])
                    return e12

                def out_mm(qt, i, kt, e12):
                    first, last = (i == 0), (i == NKT - 1)
                    o12 = o_ps[qt]
                    nc.tensor.matmul(o12[:, 0, :], v_sb[:, kt, :],
                                     e12[:, 0, :], start=first, stop=last)
                    nc.tensor.matmul(o12[:, 1, :], v_sb[:, kt, :],
                                     e12[:, 1, :], start=first, stop=last)

                def epilogue_r(qt):
                    # row-sum reduction; only needs racc, not o
                    rinvs = []
                    for h in range(2):
                        HQ = QT // 2
                        hs = slice(h * HQ, (h + 1) * HQ)
                        rtot = epi_pool.tile([128, 2, HQ], F32,
                                             tag=f"rt{h}", name="rt")
                        nc.vector.tensor_add(rtot, rpart[qt][:, :, hs],
                                             racc[qt][1][:, :, hs])
                        rall = epi_pool.tile([128, 2, HQ], F32,
                                             tag=f"ra{h}", name="ra")
                        nc.gpsimd.partition_all_reduce(
                            rall, rtot, channels=128,
                            reduce_op=bass_isa.ReduceOp.add)
                        rinv = epi_pool.tile([128, 2, HQ], F32,
                                             tag=f"ri{h}", name="ri")
                        nc.vector.reciprocal(rinv, rall)
                        rinvs.append(rinv)
                    return rinvs

                def epilogue_o(qt, rinvs):
                    # all on DVE: GPSIMD cannot touch PSUM on TRN2, and
                    # qt0's chain hides under the long qt1 P@V stretch anyway.
                    # The very last chunk (qt1, second half) runs in quarters
                    # so its first DMA overlaps the remaining DVE compute.
                    eng = nc.vector
                    o12 = o_ps[qt]
                    nch = 2
                    for h in range(nch):
                        HQ = QT // nch
                        hs = slice(h * HQ, (h + 1) * HQ)
                        rinv = rinvs[h // (nch // 2)]
                        HR = QT // 2
                        rs = slice((h * HQ) % HR, (h * HQ) % HR + HQ)
                        t1 = osb.tile([128, HQ], F32, tag=f"t1{qt}{h}",
                                      name="t1")
                        t2 = osb.tile([128, HQ], F32, tag=f"t2{qt}{h}",
                                      name="t2")
                        eng.tensor_mul(t1, o12[:, 0, hs], rinv[:, 0, rs])
                        eng.scalar_tensor_tensor(
                            out=t2, in0=o12[:, 1, hs], scalar=lam_sb,
                            in1=rinv[:, 1, rs], op0=mybir.AluOpType.mult,
                            op1=mybir.AluOpType.mult)
                        ob = osb.tile([128, HQ], F32, tag=f"ob{qt}{h}",
                                      name="ob")
                        eng.tensor_sub(ob, t1, t2)
                        q0 = qt * QT + h * HQ
                        nc.sync.dma_start(
                            out=out_t.ap()[:, q0:q0 + HQ], in_=ob)

                # ---- seq-tile loop: projections + attention, interleaved ----
                with tc.tile_pool(name="pps", bufs=1, space="PSUM") as pps:
                    for st in range(NST):
                        xt = xt_pool.tile([128, NDC, ST], F32R, tag="xt")
                        s0 = st * ST
                        nh = 8 if st == 0 else 2
                        for h in range(nh):
                            w = DIN // nh
                            nc.sync.dma_start(
                                out=xt[:, h * (NDC // nh):
                                       (h + 1) * (NDC // nh), :],
                                in_=XT_t.ap()[h * w:(h + 1) * w, s0:s0 + ST]
                                .rearrange("(c p) s -> p c s", p=128)
                                .bitcast(F32R))
                            if st == 0 and h == 2:
                                load_w("wk", 2, 2)
                            if st == 0 and h == 3:
                                load_w("wv", 0, 2)
                            if st == 0 and h == 5:
                                load_w("wv", 2, 2)
                            if st == 0 and h == 7:
                                load_w("wq", 0, 4)
                                nc.gpsimd.dma_start(
                                    out=lam_sb,
                                    in_=lam_t.ap().partition_broadcast(128))
                        vT[st] = vt_pool.tile([128, ST], F32R, tag="vT",
                                              name="vTs")

                        kf = chain_pieces("wk", kT, xt, st, pps, "pa")
                        vf = chain_pieces("wv", None, xt, st, pps, "pb")
                        qf = (chain_pieces("wq", qT, xt, st, pps, "pa")
                              if st < 2 else [])
                        vt_f = vtrans(st - 1, pps) if st >= 1 else []
                        if st == 0:
                            # K, then Q (pa reuse covered by V pieces)
                            for f in (kf[0], kf[1], kf[2], kf[3], vf[0],
                                      vf[1], qf[0], qf[1], qf[2], qf[3],
                                      vf[2], vf[3]):
                                f()
                            continue
                        if st == 1:
                            # qt0 scores (kts 0-3) only need K(st0)/Q(st0);
                            # qt1 scores wait for this tile's Q chain
                            sched = [
                                None, vt_f[0], None, qf[0], None, qf[1],
                                None, qf[2], qf[3], kf[0], kf[1],
                                'q1', kf[2], 'q1', kf[3], 'q1', vf[0],
                                'q1', vf[1], vf[2], vf[3]]
                            q0s = iter(range(4))
                            q1s = iter(range(4))
                            for f in sched:
                                if f is None:
                                    j = next(q0s)
                                    e0_live[j] = sc_exp(0, j, j)
                                elif f == 'q1':
                                    j = next(q1s)
                                    sc_exp(1, j, j)
                                else:
                                    f()
                            continue
                        # st >= 2: scores at lag 1, qt0 P@V at lag 2
                        # (qt1 P@V deferred to the tail); projection pieces
                        # fill the PE between attention units
                        filler = vt_f + kf + vf
                        fi = iter(filler)
                        next(fi, lambda: None)()
                        next(fi, lambda: None)()
                        for j in range(4):
                            kt = (st - 1) * 4 + j
                            ktp = (st - 2) * 4 + j
                            e0_live[kt] = sc_exp(0, kt, kt)
                            next(fi, lambda: None)()
                            sc_exp(1, kt, kt)
                            next(fi, lambda: None)()
                            out_mm(0, ktp, ktp, e0_live.pop(ktp))
                        for f in fi:
                            f()
                    # last V transpose
                    for f in vtrans(NST - 1, pps):
                        f()

                # ---- tail: kts 28..31 scores/exp for both qts, with the
                # deferred qt1 P@V matmuls as pure-PE filler ----
                with tc.tile_pool(name="ops1", bufs=1, space="PSUM") as ops1:
                    o_ps[1] = ops1.tile([128, 2, QT], F32, tag="o1",
                                        name="o1")
                    fi = iter(range(16))  # first deferred qt1 outs
                    for kt in range(LOOPK, NKT):
                        e0_live[kt] = sc_exp(0, kt, kt)
                        for _ in range(2):
                            k2 = next(fi, None)
                            if k2 is not None:
                                out_mm(1, k2, k2, e1_sb[:, k2, :, :])
                        sc_exp(1, kt, kt)
                        for _ in range(2):
                            k2 = next(fi, None)
                            if k2 is not None:
                                out_mm(1, k2, k2, e1_sb[:, k2, :, :])
                    # row-sum reductions run on DVE/Pool while the PE
                    # finishes the P@V matmuls
                    rinvs0 = epilogue_r(0)
                    rinvs1 = epilogue_r(1)
                    for kt in range(LOOPK - 4, NKT):
                        out_mm(0, kt, kt, e0_live.pop(kt))
                    epilogue_o(0, rinvs0)
                    # a long pure-PE stretch: every epilogue dependency
                    # (reciprocals, qt0 output chain) resolves under it
                    for kt in range(16, NKT):
                        out_mm(1, kt, kt, e1_sb[:, kt, :, :])
                    epilogue_o(1, rinvs1)

    nc.compile()
    return nc


_NC_CACHE = None


def kernel(X, W_q, W_k, W_v, lam):
    global _NC_CACHE
    from concourse.bass_utils import run_bass_kernel_spmd

    X = np.asarray(X, dtype=np.float32)
    W_q = np.asarray(W_q, dtype=np.float32)
    W_k = np.asarray(W_k, dtype=np.float32)
    W_v = np.asarray(W_v, dtype=np.float32)
    lam_arr = np.asarray(lam, dtype=np.float32).reshape(1, 1)

    if _NC_CACHE is None:
        _NC_CACHE = build_nc()
    nc = _NC_CACHE

    in_maps = []
    for c in range(8):
        b, qc = divmod(c, 4)
        qs = qc * NQ
        XTb = X[b].T  # [DIN, S]
        XTc = np.ascontiguousarray(
            np.concatenate([XTb[:, qs:], XTb[:, :qs]], axis=1))
        in_maps.append({"XT": XTc, "Wq": W_q, "Wk": W_k, "Wv": W_v,
                        "lam": lam_arr})

    res = run_bass_kernel_spmd(nc, in_maps, core_ids=list(range(8)))

    out = np.empty((B, S, TD), dtype=np.float32)
    for c in range(8):
        b, qc = divmod(c, 4)
        qs = qc * NQ
        out[b, qs:qs + NQ] = res.results[c]["out"].T
    return out


# revision 35
# speedup vs baseline: 1.0160x; 1.0123x over previous
"""DiffAttn kernel for 8 Trainium2 NeuronCores.

Problem: out = softmax(Q1 K1^T / sqrt(d)) V - lam * softmax(Q2 K2^T / sqrt(d)) V
with Q = X W_q, K = X W_k, V = X W_v;  X [2, 4096, 1024], W [1024, 128], d = 64.

Sharding: 8 cores = (batch b, query-chunk qc) with b = core // 4, qc = core % 4.
Each core receives its batch's X TRANSPOSED ([din, seq]) with the seq columns
rolled so its 1024 query columns come first (attention is permutation-invariant
over keys).  Feeding X^T from the host removes every on-device transpose of X:
projections contract din directly.  Each core computes K^T/V^T for the full
(rolled) sequence and Q^T for its 1024 queries, then two-branch flash attention
without max-subtraction (scores ~N(0,1); exp is safe), normalizing at the end.

Engine split: PE does projections + scores + P@V only.  ACT does exp
(f32 PSUM -> bf16 SBUF).  DVE accumulates the exp tiles (bf16 partials, fp32
fold) for the softmax row sums; Pool drains PSUM->SBUF and performs the final
128-partition reduction (partition_all_reduce).  Because one exp instruction
(>=1042 ns) is slower than the four PE matmuls per key tile (852 ns), pure
attention stretches would stall the PE on the 2-deep score-PSUM rotation; the
schedule therefore (a) interleaves projection-chain matmuls between attention
units inside the seq-tile loop, and (b) defers the second query-tile's P@V
matmuls to the tail, feeding them from SBUF-buffered bf16 exp tiles so the
tail is pure PE work.  Output is written transposed [2d, 1024] and fixed up on
the host.
"""

import sys

if '/opt/trn_rl_repo' not in sys.path:
    sys.path.insert(0, '/opt/trn_rl_repo')

import numpy as np

B, S, DIN, D = 2, 4096, 1024, 64
TD = 2 * D            # 128: both branches' head dims, packed on partitions
NQ = S // 4           # 1024 query rows per core
ST = 512              # seq tile (projection granularity)
NST = S // ST         # 8
QT = 512              # query tile in attention
NQT = NQ // QT        # 2
KT = 128              # key tile in attention
NKT = S // KT         # 32
NDC = DIN // 128      # 8 contraction chunks
LOOPK = 28            # key tiles scored inside the seq-tile loop (lag 1)


def build_nc():
    import concourse.bacc as bacc
    import concourse.mybir as mybir
    import concourse.bass_isa as bass_isa
    from concourse.tile import TileContext
    from concourse.masks import make_identity

    F32 = mybir.dt.float32
    F32R = mybir.dt.float32r
    BF16 = mybir.dt.bfloat16
    AF = mybir.ActivationFunctionType

    nc = bacc.Bacc("TRN2", target_bir_lowering=False)
    XT_t = nc.dram_tensor("XT", [DIN, S], F32, kind="ExternalInput")
    Wq_t = nc.dram_tensor("Wq", [DIN, TD], F32, kind="ExternalInput")
    Wk_t = nc.dram_tensor("Wk", [DIN, TD], F32, kind="ExternalInput")
    Wv_t = nc.dram_tensor("Wv", [DIN, TD], F32, kind="ExternalInput")
    lam_t = nc.dram_tensor("lam", [1, 1], F32, kind="ExternalInput")
    out_t = nc.dram_tensor("out", [TD, NQ], F32, kind="ExternalOutput")

    with TileContext(nc) as tc:
        with tc.tile_pool(name="consts", bufs=1) as consts, \
             tc.tile_pool(name="kv", bufs=1) as kv:
            w_sb = {}
            w_dram = {"wk": Wk_t, "wv": Wv_t, "wq": Wq_t}
            for name in ("wk", "wv", "wq"):
                w_sb[name] = consts.tile([128, NDC, TD], F32R, tag=name,
                                         name=name)

            def load_w(name, q, n=1):
                qc = NDC // 4
                nc.sync.dma_start(
                    out=w_sb[name][:, q * qc:(q + n) * qc, :],
                    in_=w_dram[name].ap()[q * 256:(q + n) * 256, :]
                    .rearrange("(c p) n -> p c n", p=128).bitcast(F32R))
            load_w("wk", 0, 2)
            ident = consts.tile([128, 128], F32, tag="ident")
            make_identity(nc, ident)
            identr = consts.tile([128, 128], F32R, tag="identr")
            nc.scalar.copy(out=identr, in_=ident)
            lam_sb = consts.tile([128, 1], F32, tag="lam")

            # resident projections + the qt1 exp-tile store
            kT = kv.tile([128, S], F32R, tag="kT")         # K^T [2d, S]
            qT = kv.tile([128, NQ], F32R, tag="qT")        # Q^T [2d, NQ]
            v_sb = kv.tile([128, NKT, TD], BF16, tag="v")  # V natural
            e1_sb = kv.tile([128, NKT, 2, QT], BF16, tag="e1")  # qt1 exps

            with tc.tile_pool(name="xt", bufs=2) as xt_pool, \
                 tc.tile_pool(name="vts", bufs=2) as vt_pool, \
                 tc.tile_pool(name="sps", bufs=2, space="PSUM") as sps, \
                 tc.tile_pool(name="ops0", bufs=1, space="PSUM") as ops0, \
                 tc.tile_pool(name="e0", bufs=8) as e0_pool, \
                 tc.tile_pool(name="racc", bufs=1) as racc_pool, \
                 tc.tile_pool(name="epi", bufs=1) as epi_pool, \
                 tc.tile_pool(name="osb", bufs=1) as osb:

                racc = [[racc_pool.tile([128, 2, QT], BF16, tag=f"a{q}{j}",
                                        name=f"a{q}{j}")
                         for j in range(2)] for q in range(NQT)]
                rpart = [racc_pool.tile([128, 2, QT], F32, tag=f"rp{q}",
                                        name=f"rp{q}") for q in range(NQT)]
                o_ps = [None, None]
                o_ps[0] = (ops0.tile([128, QT], F32, tag="o0a", name="o0a"),
                           ops0.tile([128, QT], F32, tag="o0b", name="o0b"))
                e0_live = {}
                vT = [None] * NST

                def chain_pieces(wname, dst, xt, st, pps, tag):
                    """Projection chain for one seq tile as 4 interleavable
                    2-matmul pieces; the final piece issues the Pool copy."""
                    ps = pps.tile([128, ST], F32, tag=tag, name="ps")

                    def piece(i):
                        for dc in (2 * i, 2 * i + 1):
                            nc.tensor.matmul(
                                ps, w_sb[wname][:, dc, :], xt[:, dc, :],
                                start=(dc == 0), stop=(dc == NDC - 1))
                        if i == 3:
                            nc.vector.tensor_copy(
                                dst[:, st * ST:(st + 1) * ST]
                                if dst is not None else vT[st], ps)
                    return [lambda i=i: piece(i) for i in range(4)]

                def vtrans(st, pps):
                    # V natural blocks for seq tile st
                    def piece():
                        vp4 = pps.tile([128, 4, TD], F32R, tag="pb",
                                       name="vp4")
                        for j in range(4):
                            nc.tensor.transpose(
                                vp4[:, j, :], vT[st][:, j * 128:(j + 1) * 128],
                                identr)
                        nc.vector.tensor_copy(v_sb[:, st * 4:st * 4 + 4, :],
                                               vp4)
                    return [piece]

                def sc_exp(qt, i, kt):
                    """Scores + exp (+ row-sum add) for one (qt, key tile)."""
                    q0 = qt * QT
                    k0 = kt * KT
                    s12 = sps.tile([128, 2, QT], F32, tag="s", name="s12")
                    nc.tensor.matmul(
                        s12[:, 0, :], kT[0:64, k0:k0 + KT],
                        qT[0:64, q0:q0 + QT], start=True, stop=True)
                    nc.tensor.matmul(
                        s12[:, 1, :], kT[64:128, k0:k0 + KT],
                        qT[64:128, q0:q0 + QT], start=True, stop=True)
                    if qt == 0:
                        e12 = e0_pool.tile([128, 2, QT], BF16, tag="e")
                    else:
                        e12 = e1_sb[:, kt, :, :]
                    nc.scalar.activation(
                        out=e12, in_=s12, func=AF.Exp, scale=0.125)
                    # block-grouped row-sum accumulation: 4 groups of 8
                    # key tiles in 2 alternating bf16 tiles, folded into the
                    # fp32 partial as each group completes, so only one fold
                    # remains after the last exp
                    a = racc[qt][(i // 8) % 2]
                    if i % 8 == 0:
                        nc.vector.tensor_copy(a, e12)
                    else:
                        nc.vector.tensor_add(a, a, e12)
                    if i == 15:
                        nc.vector.tensor_add(rpart[qt], racc[qt][0],
                                             racc[qt][1])
                    elif i == 23:
                        nc.vector.tensor_add(rpart[qt], rpart[qt],
                                             racc[qt][0])
                    return e12

                def out_mm(qt, i, kt, e12):
                    first, last = (i == 0), (i == NKT - 1)
                    oa, ob_ = o_ps[qt]
                    nc.tensor.matmul(oa, v_sb[:, kt, :],
                                     e12[:, 0, :], start=first, stop=last)
                    nc.tensor.matmul(ob_, v_sb[:, kt, :],
                                     e12[:, 1, :], start=first, stop=last)

                def epilogue_r(qt):
                    # row-sum reduction; only needs racc, not o
                    rinvs = []
                    for h in range(2):
                        HQ = QT // 2
                        hs = slice(h * HQ, (h + 1) * HQ)
                        rtot = epi_pool.tile([128, 2, HQ], F32,
                                             tag=f"rt{h}", name="rt")
                        nc.vector.tensor_add(rtot, rpart[qt][:, :, hs],
                                             racc[qt][1][:, :, hs])
                        rall = epi_pool.tile([128, 2, HQ], F32,
                                             tag=f"ra{h}", name="ra")
                        nc.gpsimd.partition_all_reduce(
                            rall, rtot, channels=128,
                            reduce_op=bass_isa.ReduceOp.add)
                        rinv = epi_pool.tile([128, 2, HQ], F32,
                                             tag=f"ri{h}", name="ri")
                        nc.vector.reciprocal(rinv, rall)
                        rinvs.append(rinv)
                    return rinvs

                def epilogue_o(qt, rinvs):
                    # all on DVE: GPSIMD cannot touch PSUM on TRN2, and
                    # qt0's chain hides under the long qt1 P@V stretch anyway.
                    # The very last chunk (qt1, second half) runs in quarters
                    # so its first DMA overlaps the remaining DVE compute.
                    eng = nc.vector
                    oa, ob_ = o_ps[qt]
                    HQ = QT // 2
                    t1s = []
                    for h in range(2):
                        hs = slice(h * HQ, (h + 1) * HQ)
                        t1 = osb.tile([128, HQ], F32, tag=f"t1{qt}{h}",
                                      name="t1")
                        eng.tensor_mul(t1, oa[:, hs], rinvs[h][:, 0, :])
                        t1s.append(t1)
                    for h in range(2):
                        hs = slice(h * HQ, (h + 1) * HQ)
                        t2 = osb.tile([128, HQ], F32, tag=f"t2{qt}{h}",
                                      name="t2")
                        eng.scalar_tensor_tensor(
                            out=t2, in0=ob_[:, hs], scalar=lam_sb,
                            in1=rinvs[h][:, 1, :], op0=mybir.AluOpType.mult,
                            op1=mybir.AluOpType.mult)
                        ob = osb.tile([128, HQ], F32, tag=f"ob{qt}{h}",
                                      name="ob")
                        eng.tensor_sub(ob, t1s[h], t2)
                        q0 = qt * QT + h * HQ
                        nc.sync.dma_start(
                            out=out_t.ap()[:, q0:q0 + HQ], in_=ob)

                # ---- seq-tile loop: projections + attention, interleaved ----
                with tc.tile_pool(name="pps", bufs=1, space="PSUM") as pps:
                    for st in range(NST):
                        xt = xt_pool.tile([128, NDC, ST], F32R, tag="xt")
                        s0 = st * ST
                        nh = 8 if st == 0 else 2
                        for h in range(nh):
                            w = DIN // nh
                            nc.sync.dma_start(
                                out=xt[:, h * (NDC // nh):
                                       (h + 1) * (NDC // nh), :],
                                in_=XT_t.ap()[h * w:(h + 1) * w, s0:s0 + ST]
                                .rearrange("(c p) s -> p c s", p=128)
                                .bitcast(F32R))
                            if st == 0 and h == 2:
                                load_w("wk", 2, 2)
                            if st == 0 and h == 2:
                                load_w("wv", 0, 2)
                            if st == 0 and h == 4:
                                load_w("wv", 2, 2)
                            if st == 0 and h == 7:
                                load_w("wq", 0, 4)
                                nc.gpsimd.dma_start(
                                    out=lam_sb,
                                    in_=lam_t.ap().partition_broadcast(128))
                        vT[st] = vt_pool.tile([128, ST], F32R, tag="vT",
                                              name="vTs")

                        kf = chain_pieces("wk", kT, xt, st, pps, "pa")
                        vf = chain_pieces("wv", None, xt, st, pps, "pb")
                        qf = (chain_pieces("wq", qT, xt, st, pps, "pa")
                              if st < 2 else [])
                        vt_f = vtrans(st - 1, pps) if st >= 1 else []
                        if st == 0:
                            # K, then Q (pa reuse covered by V pieces)
                            for f in (kf[0], kf[1], kf[2], kf[3], vf[0],
                                      vf[1], qf[0], qf[1], qf[2], qf[3],
                                      vf[2], vf[3]):
                                f()
                            continue
                        if st == 1:
                            # qt0 scores (kts 0-3) only need K(st0)/Q(st0);
                            # qt1 scores wait for this tile's Q chain
                            sched = [
                                None, vt_f[0], None, kf[0], None, kf[1],
                                None, kf[2], kf[3], qf[0], qf[1], qf[2],
                                qf[3], vf[0], vf[1], 'q1', vf[2],
                                'q1', vf[3], 'q1', 'q1']
                            q0s = iter(range(4))
                            q1s = iter(range(4))
                            for f in sched:
                                if f is None:
                                    j = next(q0s)
                                    e0_live[j] = sc_exp(0, j, j)
                                elif f == 'q1':
                                    j = next(q1s)
                                    sc_exp(1, j, j)
                                else:
                                    f()
                            continue
                        # st >= 2: scores at lag 1, qt0 P@V at lag 2
                        # (qt1 P@V deferred to the tail); projection pieces
                        # fill the PE between attention units
                        filler = vt_f + kf + vf
                        fi = iter(filler)
                        next(fi, lambda: None)()
                        next(fi, lambda: None)()
                        for j in range(4):
                            kt = (st - 1) * 4 + j
                            ktp = (st - 2) * 4 + j
                            e0_live[kt] = sc_exp(0, kt, kt)
                            next(fi, lambda: None)()
                            sc_exp(1, kt, kt)
                            next(fi, lambda: None)()
                            out_mm(0, ktp, ktp, e0_live.pop(ktp))
                        for f in fi:
                            f()
                    # last V transpose
                    for f in vtrans(NST - 1, pps):
                        f()

                # ---- tail: kts 28..31 scores/exp for both qts, with the
                # deferred qt1 P@V matmuls as pure-PE filler ----
                with tc.tile_pool(name="ops1", bufs=1, space="PSUM") as ops1:
                    o_ps[1] = (ops1.tile([128, QT], F32, tag="o1a",
                                         name="o1a"),
                               ops1.tile([128, QT], F32, tag="o1b",
                                         name="o1b"))
                    fi = iter(range(16))  # first deferred qt1 outs
                    for kt in range(LOOPK, NKT):
                        e0_live[kt] = sc_exp(0, kt, kt)
                        for _ in range(2):
                            k2 = next(fi, None)
                            if k2 is not None:
                                out_mm(1, k2, k2, e1_sb[:, k2, :, :])
                        sc_exp(1, kt, kt)
                        for _ in range(2):
                            k2 = next(fi, None)
                            if k2 is not None:
                                out_mm(1, k2, k2, e1_sb[:, k2, :, :])
                    # row-sum reductions run on DVE/Pool while the PE
                    # finishes the P@V matmuls
                    rinvs0 = epilogue_r(0)
                    rinvs1 = epilogue_r(1)
                    for kt in range(LOOPK - 4, NKT):
                        out_mm(0, kt, kt, e0_live.pop(kt))
                    epilogue_o(0, rinvs0)
                    # a long pure-PE stretch: every epilogue dependency
                    # (reciprocals, qt0 output chain) resolves under it
                    # final stretch split by branch: branch 0 closes its
                    # accumulation early so the epilogue's o1-multiplies run
                    # under the remaining branch-1 matmuls
                    for br in range(2):
                        for kt in range(16, NKT):
                            nc.tensor.matmul(
                                o_ps[1][br], v_sb[:, kt, :],
                                e1_sb[:, kt, br, :], start=False,
                                stop=(kt == NKT - 1))
                    epilogue_o(1, rinvs1)

    nc.compile()
    return nc


_NC_CACHE = None


def kernel(X, W_q, W_k, W_v, lam):
    global _NC_CACHE
    from concourse.bass_utils import run_bass_kernel_spmd

    X = np.asarray(X, dtype=np.float32)
    W_q = np.asarray(W_q, dtype=np.float32)
    W_k = np.asarray(W_k, dtype=np.float32)
    W_v = np.asarray(W_v, dtype=np.float32)
    lam_arr = np.asarray(lam, dtype=np.float32).reshape(1, 1)

    if _NC_CACHE is None:
        _NC_CACHE = build_nc()
    nc = _NC_CACHE

    in_maps = []
    for c in range(8):
        b, qc = divmod(c, 4)
        qs = qc * NQ
        XTb = X[b].T  # [DIN, S]
        XTc = np.ascontiguousarray(
            np.concatenate([XTb[:, qs:], XTb[:, :qs]], axis=1))
        in_maps.append({"XT": XTc, "Wq": W_q, "Wk": W_k, "Wv": W_v,
                        "lam": lam_arr})

    res = run_bass_kernel_spmd(nc, in_maps, core_ids=list(range(8)))

    out = np.empty((B, S, TD), dtype=np.float32)
    for c in range(8):
        b, qc = divmod(c, 4)
        qs = qc * NQ
        out[b, qs:qs + NQ] = res.results[c]["out"].T
    return out
